# revision 8
# baseline (speedup 1.0000x reference)
import sys
sys.path.insert(0, '/opt/trn_rl_repo')
import numpy as np
from contextlib import ExitStack

N = 65536
NCORES = 8
NC_SAMP = N // NCORES          # 8192 samples per core
F = NC_SAMP // 128             # 64 samples per partition
NCHUNK = 16                    # matmul chunks of 512 samples
Q10 = 10                       # bundle rows: sp0,sp1,j00,j10,j01,j11,z0,z1,r0,r1

_built = {}


def _build(reps=1):
    if reps in _built:
        return _built[reps]
    import concourse.bass as bass
    import concourse.tile as tile
    from concourse import bacc, mybir

    dt = mybir.dt
    F32, F32R = dt.float32, dt.float32r
    AF = mybir.ActivationFunctionType
    AL = mybir.AluOpType

    nc = bacc.Bacc("TRN2", target_bir_lowering=False, debug=False,
                   enable_asserts=False, num_devices=NCORES)

    def din(name, shape):
        return nc.dram_tensor(name, shape, F32, kind="ExternalInput").ap()

    x1_d = din("x1", [128, F * 4])
    obs_d = din("obs", [128, F * 64])
    sg_d = din("sg", [128, F * 4])
    wd1_d = din("wd1", [4, 256]);   wd2_d = din("wd2", [128, 512])
    wd3_d = din("wd3", [128, 4]);   wm1_d = din("wm1", [66, 256])
    wm2_d = din("wm2", [128, 512]); wzr_d = din("wzr", [128, 8])
    b1_d = din("b1", [128, 2]);     b2_d = din("b2", [128, 2])
    bm1_d = din("bm1", [128, 2]);   bm2_d = din("bm2", [128, 2])
    bd3_d = din("bd3", [2, 1]);     bz2_d = din("bz2", [2, 1]);     br2_d = din("br2", [2, 1])
    w1r_d = din("w1r", [128, 4]);   qb_d = din("qb", [128, 4])
    id_d = din("ident", [128, 128])
    ost_d = nc.dram_tensor("o_st", [128, F * 2], F32, kind="ExternalOutput").ap()
    osg_d = nc.dram_tensor("o_sg", [128, F * 4], F32, kind="ExternalOutput").ap()

    with tile.TileContext(nc) as tc, ExitStack() as ctx:
        wp = ctx.enter_context(tc.tile_pool(name="w", bufs=1))
        iop = ctx.enter_context(tc.tile_pool(name="io", bufs=1))
        obp = ctx.enter_context(tc.tile_pool(name="ob", bufs=3))
        wk = ctx.enter_context(tc.tile_pool(name="wk", bufs=2))
        ek = ctx.enter_context(tc.tile_pool(name="ek", bufs=2))
        pp = ctx.enter_context(tc.tile_pool(name="pp", bufs=8, space="PSUM"))

        def wload(dram, shape, cvt=True):
            t = wp.tile(shape, F32, tag="wf_" + dram.tensor.name)
            nc.sync.dma_start(t[:], dram[:])
            if not cvt:
                return t
            r = wp.tile(shape, F32R, tag="wr_" + dram.tensor.name)
            nc.vector.tensor_copy(r[:], t[:])
            return r

        wd1 = wload(wd1_d, [4, 256], False)
        wd2 = wload(wd2_d, [128, 512], False)

        wd3 = wload(wd3_d, [128, 4], False)
        wm1 = wload(wm1_d, [66, 256], False)
        wm2 = wload(wm2_d, [128, 512], False)
        wzr = wload(wzr_d, [128, 8], False)
        b1 = wload(b1_d, [128, 2], False);   b2 = wload(b2_d, [128, 2], False)
        bm1 = wload(bm1_d, [128, 2], False); bm2 = wload(bm2_d, [128, 2], False)
        bd3 = wload(bd3_d, [2, 1], False)
        bz2 = wload(bz2_d, [2, 1], False);   br2 = wload(br2_d, [2, 1], False)
        w1r = wload(w1r_d, [128, 4], False); qb = wload(qb_d, [128, 4], False)
        ident = wload(id_d, [128, 128], False)

        x1_sm = iop.tile([128, F * 4], F32)
        nc.sync.dma_start(x1_sm[:], x1_d[:])
        sg_sm = iop.tile([128, F * 4], F32)
        nc.sync.dma_start(sg_sm[:], sg_d[:])
        stag = iop.tile([128, F * Q10], F32)
        ost = iop.tile([128, F * 2], F32)
        osg = iop.tile([128, F * 4], F32)

        def chunk(m):
            obs_t = obp.tile([128, 256], F32, tag="obs")
            nc.sync.dma_start(obs_t[:], obs_d[:, m * 256:(m + 1) * 256])
            x1tp = pp.tile([4, 512], F32, tag="ps")
            obtp = pp.tile([64, 512], F32, tag="ps")
            for k in range(4):
                c = 4 * m + k
                nc.tensor.transpose(x1tp[:, k * 128:(k + 1) * 128],
                                    x1_sm[:, c * 4:(c + 1) * 4], ident[:])
                nc.tensor.transpose(obtp[:, k * 128:(k + 1) * 128],
                                    obs_t[:, k * 64:(k + 1) * 64], ident[:])
            x1t = wk.tile([4, 512], F32, tag="x1t")
            nc.vector.tensor_copy(x1t[:], x1tp[:])
            mrhs = wk.tile([66, 512], F32, tag="mrhs")
            nc.scalar.copy(mrhs[0:64, :], obtp[:])

            h1, mk1 = [], []
            for t in range(2):
                p = pp.tile([128, 512], F32, tag="ps")
                nc.tensor.matmul(p[:], wd1[:, t * 128:(t + 1) * 128], x1t[:],
                                 start=True, stop=True)
                h = wk.tile([128, 512], F32, tag=f"h1_{t}")
                nc.scalar.activation(h[:], p[:], AF.Relu, bias=b1[:, t:t + 1])
                mk = wk.tile([128, 512], F32, tag=f"mk1_{t}")
                nc.vector.tensor_scalar(mk[:], h[:], 0.0, None, AL.is_gt)
                h1.append(h); mk1.append(mk)

            h2, mk2 = [], []
            for u in range(2):
                p = pp.tile([128, 512], F32, tag="ps")
                for kt in range(2):
                    nc.tensor.matmul(p[:], wd2[:, kt * 256 + u * 128: kt * 256 + (u + 1) * 128],
                                     h1[kt][:], start=(kt == 0), stop=(kt == 1))
                h = wk.tile([128, 512], F32, tag=f"h2_{u}")
                nc.scalar.activation(h[:], p[:], AF.Relu, bias=b2[:, u:u + 1])
                mk = wk.tile([128, 512], F32, tag=f"mk2_{u}")
                nc.vector.tensor_scalar(mk[:], h[:], 0.0, None, AL.is_gt)
                h2.append(h); mk2.append(mk)

            d3p = pp.tile([2, 512], F32, tag="ps")
            for kt in range(2):
                nc.tensor.matmul(d3p[:], wd3[:, kt * 2:(kt + 1) * 2], h2[kt][:],
                                 start=(kt == 0), stop=(kt == 1))
            bnd = wk.tile([Q10, 512], F32, tag="bnd")
            nc.vector.scalar_tensor_tensor(bnd[0:2, :], d3p[:], bd3[:], x1t[0:2, :],
                                           AL.add, AL.add)
            nc.scalar.copy(mrhs[64:66, :], bnd[0:2, :])

            for i in range(2):
                t1 = []
                for t in range(2):
                    tt = wk.tile([128, 512], F32, tag=f"t1_{i}_{t}")
                    nc.vector.tensor_scalar(tt[:], mk1[t][:], w1r[:, 2 * i + t:2 * i + t + 1],
                                            None, AL.mult)
                    t1.append(tt)
                t2 = []
                for u in range(2):
                    p = pp.tile([128, 512], F32, tag="ps")
                    for kt in range(2):
                        nc.tensor.matmul(p[:], wd2[:, kt * 256 + u * 128: kt * 256 + (u + 1) * 128],
                                         t1[kt][:], start=(kt == 0), stop=(kt == 1))
                    tt2 = wk.tile([128, 512], F32, tag=f"t2_{i}_{u}")
                    nc.vector.tensor_tensor(tt2[:], p[:], mk2[u][:], AL.mult)
                    t2.append(tt2)
                jp = pp.tile([2, 512], F32, tag="ps")
                for kt in range(2):
                    nc.tensor.matmul(jp[:], wd3[:, kt * 2:(kt + 1) * 2], t2[kt][:],
                                     start=(kt == 0), stop=(kt == 1))
                jq = wk.tile([2, 512], F32, tag=f"jq_{i}", name=f"jq_{i}")
                nc.vector.tensor_copy(jq[:], jp[:])
                nc.sync.dma_start(bnd[2 + 2 * i:4 + 2 * i, :], jq[:])

            g1 = []
            for t in range(2):
                p = pp.tile([128, 512], F32, tag="ps")
                nc.tensor.matmul(p[:], wm1[:, t * 128:(t + 1) * 128], mrhs[:],
                                 start=True, stop=True)
                g = wk.tile([128, 512], F32, tag=f"g1_{t}")
                nc.scalar.activation(g[:], p[:], AF.Relu, bias=bm1[:, t:t + 1])
                g1.append(g)
            g2 = []
            for u in range(2):
                p = pp.tile([128, 512], F32, tag="ps")
                for kt in range(2):
                    nc.tensor.matmul(p[:], wm2[:, kt * 256 + u * 128: kt * 256 + (u + 1) * 128],
                                     g1[kt][:], start=(kt == 0), stop=(kt == 1))
                g = wk.tile([128, 512], F32, tag=f"g2_{u}")
                nc.scalar.activation(g[:], p[:], AF.Relu, bias=bm2[:, u:u + 1])
                g2.append(g)
            zp = pp.tile([2, 512], F32, tag="ps")
            rp = pp.tile([2, 512], F32, tag="ps")
            for kt in range(2):
                nc.tensor.matmul(zp[:], wzr[:, kt * 4:kt * 4 + 2], g2[kt][:],
                                 start=(kt == 0), stop=(kt == 1))
            for kt in range(2):
                nc.tensor.matmul(rp[:], wzr[:, kt * 4 + 2:(kt + 1) * 4], g2[kt][:],
                                 start=(kt == 0), stop=(kt == 1))
            zq = wk.tile([2, 512], F32, tag="zq")
            nc.vector.tensor_scalar(zq[:], zp[:], bz2[:], None, AL.add)
            nc.sync.dma_start(bnd[6:8, :], zq[:])
            # r = softplus(pre_r)+1e-3, pre_r = rp+br; softplus = max(x,0)+ln(1+exp(-|x|))
            axt = wk.tile([2, 512], F32, tag="axt")
            nc.scalar.activation(axt[:], rp[:], AF.Abs, bias=br2[:])
            ext = wk.tile([2, 512], F32, tag="ext")
            nc.scalar.activation(ext[:], axt[:], AF.Exp, scale=-1.0)
            lnt = wk.tile([2, 512], F32, tag="lnt")
            nc.scalar.activation(lnt[:], ext[:], AF.Ln, bias=1.0)
            mxt = wk.tile([2, 512], F32, tag="mxt")
            nc.vector.tensor_scalar(mxt[:], rp[:], br2[:], 0.0, AL.add, AL.max)
            rq = wk.tile([2, 512], F32, tag="rq")
            nc.vector.scalar_tensor_tensor(rq[:], lnt[:], 1e-3, mxt[:], AL.add, AL.add)
            nc.sync.dma_start(bnd[8:10, :], rq[:])

            bndt = pp.tile([128, 4 * Q10], F32, tag="ps")
            for k in range(4):
                nc.tensor.transpose(bndt[:, k * Q10:(k + 1) * Q10],
                                    bnd[:, k * 128:(k + 1) * 128], ident[0:Q10, 0:Q10])
            nc.vector.tensor_copy(stag[:, m * 4 * Q10:(m + 1) * 4 * Q10], bndt[:])

        def ekf():
            stv = stag[:].rearrange("p (c q) -> p c q", q=Q10)
            sgv = sg_sm[:].rearrange("p (f e) -> p f e", e=4)
            ostv = ost[:].rearrange("p (c l) -> p c l", l=2)
            osgv = osg[:].rearrange("p (c l) -> p c l", l=4)
            S = lambda q: stv[:, :, q:q + 1]
            P = lambda e: sgv[:, :, e:e + 1]
            sc = {}

            def T(name):
                if name not in sc:
                    sc[name] = ek.tile([128, F, 1], F32, tag="e_" + name, name="e_" + name)
                return sc[name][:]

            V = nc.vector
            sp0, sp1 = S(0), S(1)
            j00, j10, j01, j11 = S(2), S(3), S(4), S(5)
            z0, z1, r0, r1 = S(6), S(7), S(8), S(9)
            p00, p01, p10, p11 = P(0), P(1), P(2), P(3)
            a00, a11 = T("a00"), T("a11")
            V.tensor_scalar(a00, j00, 1.0, None, AL.add)
            V.tensor_scalar(a11, j11, 1.0, None, AL.add)
            t, u = T("t"), T("u")
            # AP = A @ P
            ap00, ap01, ap10, ap11 = T("ap00"), T("ap01"), T("ap10"), T("ap11")
            for (o, aL, pL, aR, pR) in ((ap00, a00, p00, j01, p10), (ap01, a00, p01, j01, p11),
                                        (ap10, j10, p00, a11, p10), (ap11, j10, p01, a11, p11)):
                V.tensor_tensor(t, aL, pL, AL.mult)
                V.tensor_tensor(u, aR, pR, AL.mult)
                V.tensor_tensor(o, t, u, AL.add)
            # Sigma_pred = AP @ A^T + Q
            s00, s01, s10, s11 = T("s00"), T("s01"), T("s10"), T("s11")
            for (o, x1_, aL, x2_, aR, qi) in (
                    (s00, ap00, a00, ap01, j01, 0), (s01, ap00, j10, ap01, a11, 1),
                    (s10, ap10, a00, ap11, j01, 2), (s11, ap10, j10, ap11, a11, 3)):
                V.tensor_tensor(t, x1_, aL, AL.mult)
                V.tensor_tensor(u, x2_, aR, AL.mult)
                V.scalar_tensor_tensor(o, t, qb[:, qi:qi + 1], u, AL.add, AL.add)
            # S = Sigma + R (diag), inverse via adjugate
            e00, e11 = T("e00"), T("e11")
            V.tensor_tensor(e00, s00, r0, AL.add)
            V.tensor_tensor(e11, s11, r1, AL.add)
            det, rdet = T("det"), T("rdet")
            V.tensor_tensor(t, e00, e11, AL.mult)
            V.tensor_tensor(u, s01, s10, AL.mult)
            V.tensor_tensor(det, t, u, AL.subtract)
            V.reciprocal(rdet, det)
            # K = Sigma @ inv(S)
            k00, k01, k10, k11 = T("k00"), T("k01"), T("k10"), T("k11")
            for (o, x1_, aL, x2_, aR) in (
                    (k00, s00, e11, s01, s10), (k01, s01, e00, s00, s01),
                    (k10, s10, e11, s11, s10), (k11, s11, e00, s10, s01)):
                V.tensor_tensor(t, x1_, aL, AL.mult)
                V.tensor_tensor(u, x2_, aR, AL.mult)
                V.tensor_tensor(o, t, u, AL.subtract)
                V.tensor_tensor(o, o, rdet, AL.mult)
            y0, y1 = T("y0"), T("y1")
            V.tensor_tensor(y0, z0, sp0, AL.subtract)
            V.tensor_tensor(y1, z1, sp1, AL.subtract)
            for (oc, ka, kb, spx) in ((0, k00, k01, sp0), (1, k10, k11, sp1)):
                V.tensor_tensor(t, ka, y0, AL.mult)
                V.tensor_tensor(u, kb, y1, AL.mult)
                V.tensor_tensor(t, t, u, AL.add)
                V.tensor_tensor(ostv[:, :, oc:oc + 1], spx, t, AL.add)
            ik00, ik01, ik10, ik11 = T("ik00"), T("ik01"), T("ik10"), T("ik11")
            V.tensor_scalar(ik00, k00, -1.0, 1.0, AL.mult, AL.add)
            V.tensor_scalar(ik11, k11, -1.0, 1.0, AL.mult, AL.add)
            V.tensor_scalar(ik01, k01, -1.0, None, AL.mult)
            V.tensor_scalar(ik10, k10, -1.0, None, AL.mult)
            # Sigma_update = (I-K) @ Sigma ; out[0,1] forced = out[1,0]
            su10 = T("su10")
            for (o, ka, xa, kb, xb) in (
                    (osgv[:, :, 0:1], ik00, s00, ik01, s10),
                    (su10, ik10, s00, ik11, s10),
                    (osgv[:, :, 3:4], ik10, s01, ik11, s11)):
                V.tensor_tensor(t, ka, xa, AL.mult)
                V.tensor_tensor(u, kb, xb, AL.mult)
                V.tensor_tensor(o, t, u, AL.add)
            V.tensor_copy(osgv[:, :, 1:2], su10)
            V.tensor_copy(osgv[:, :, 2:3], su10)

        def body():
            for m in range(NCHUNK):
                chunk(m)
            ekf()
            nc.sync.dma_start(ost_d[:], ost[:])
            nc.sync.dma_start(osg_d[:], osg[:])

        if reps == 1:
            body()
        else:
            with tc.For_i(0, reps, 1):
                body()

    nc.compile()
    _built[reps] = nc
    return nc


def _prep_shared(Wd1, bd1, Wd2, bd2, Wd3, bd3, Wm1, bm1, Wm2, bm2, Wz, bz, Wr, br, Q):
    f = np.float32
    out = {
        "wd1": np.ascontiguousarray(Wd1, f),
        "wd2": np.ascontiguousarray(Wd2.reshape(2, 128, 256).transpose(1, 0, 2).reshape(128, 512), f),
        "wd3": np.ascontiguousarray(Wd3.reshape(2, 128, 2).transpose(1, 0, 2).reshape(128, 4), f),
        "wm1": np.ascontiguousarray(Wm1, f),
        "wm2": np.ascontiguousarray(Wm2.reshape(2, 128, 256).transpose(1, 0, 2).reshape(128, 512), f),
        "wzr": np.ascontiguousarray(np.concatenate([Wz, Wr], 1).reshape(2, 128, 4).transpose(1, 0, 2).reshape(128, 8), f),
        "b1": np.ascontiguousarray(bd1.reshape(2, 128).T, f),
        "b2": np.ascontiguousarray(bd2.reshape(2, 128).T, f),
        "bm1": np.ascontiguousarray(bm1.reshape(2, 128).T, f),
        "bm2": np.ascontiguousarray(bm2.reshape(2, 128).T, f),
        "bd3": np.ascontiguousarray(bd3.reshape(2, 1), f),
        "bz2": np.ascontiguousarray(bz.reshape(2, 1), f),
        "br2": np.ascontiguousarray(br.reshape(2, 1), f),
        "w1r": np.ascontiguousarray(
            np.stack([Wd1[0, :128], Wd1[0, 128:], Wd1[1, :128], Wd1[1, 128:]], 1), f),
        "qb": np.ascontiguousarray(np.tile(np.asarray(Q, f).reshape(1, 4), (128, 1))),
        "ident": np.eye(128, dtype=f),
    }
    return out


def run(reps, states_prev, states_sigma_prev, observations, controls,
        Wd1, bd1, Wd2, bd2, Wd3, bd3, Wm1, bm1, Wm2, bm2, Wz, bz, Wr, br, Q):
    from concourse import bass_utils
    nc = _build(reps)
    shared = _prep_shared(Wd1, bd1, Wd2, bd2, Wd3, bd3, Wm1, bm1, Wm2, bm2, Wz, bz, Wr, br, Q)
    f = np.float32
    sp = np.asarray(states_prev, f)
    sg = np.asarray(states_sigma_prev, f).reshape(N, 4)
    ob = np.asarray(observations, f)
    ct = np.asarray(controls, f)
    in_maps = []
    for c in range(NCORES):
        lo, hi = c * NC_SAMP, (c + 1) * NC_SAMP
        m = dict(shared)
        m["x1"] = np.ascontiguousarray(
            np.concatenate([sp[lo:hi], ct[lo:hi]], 1).reshape(128, F * 4))
        m["obs"] = np.ascontiguousarray(ob[lo:hi].reshape(128, F * 64))
        m["sg"] = np.ascontiguousarray(sg[lo:hi].reshape(128, F * 4))
        in_maps.append(m)
    import time
    t0 = time.time()
    res = bass_utils.run_bass_kernel_spmd(nc, in_maps, core_ids=list(range(NCORES)), trace=False)
    wall = time.time() - t0
    st = np.concatenate([res.results[c]["o_st"].reshape(NC_SAMP, 2) for c in range(NCORES)], 0)
    sgo = np.concatenate([res.results[c]["o_sg"].reshape(NC_SAMP, 2, 2) for c in range(NCORES)], 0)
    return (st, sgo), wall


def kernel(**inputs):
    (st, sgo), _ = run(1, **inputs)
    return st, sgo


# revision 9
# speedup vs baseline: 8.5261x; 8.5261x over previous
import sys
sys.path.insert(0, '/opt/trn_rl_repo')
import numpy as np
from contextlib import ExitStack

N = 65536
NCORES = 8
NC_SAMP = N // NCORES          # 8192 samples per core
F = NC_SAMP // 128             # 64 samples per partition
NCHUNK = 16                    # matmul chunks of 512 samples
Q10 = 10                       # bundle rows: sp0,sp1,j00,j10,j01,j11,z0,z1,r0,r1

_built = {}


def _build(reps=1):
    if reps in _built:
        return _built[reps]
    import concourse.bass as bass
    import concourse.tile as tile
    from concourse import bacc, mybir

    dt = mybir.dt
    F32, F32R = dt.float32, dt.float32r
    AF = mybir.ActivationFunctionType
    AL = mybir.AluOpType

    nc = bacc.Bacc("TRN2", target_bir_lowering=False, debug=False,
                   enable_asserts=False, num_devices=NCORES)

    def din(name, shape):
        return nc.dram_tensor(name, shape, F32, kind="ExternalInput").ap()

    x1_d = din("x1", [128, F * 4])
    obs_d = din("obs", [128, F * 64])
    sg_d = din("sg", [128, F * 4])
    wd1_d = din("wd1", [4, 256]);   wd2_d = din("wd2", [128, 512])
    wd3_d = din("wd3", [128, 4]);   wm1_d = din("wm1", [66, 256])
    wm2_d = din("wm2", [128, 512]); wzr_d = din("wzr", [128, 8])
    b1_d = din("b1", [128, 2]);     b2_d = din("b2", [128, 2])
    bm1_d = din("bm1", [128, 2]);   bm2_d = din("bm2", [128, 2])
    bd3_d = din("bd3", [2, 1]);     bz2_d = din("bz2", [2, 1]);     br2_d = din("br2", [2, 1])
    w1r_d = din("w1r", [128, 4]);   qb_d = din("qb", [128, 4])
    id_d = din("ident", [128, 128])
    ost_d = nc.dram_tensor("o_st", [128, F * 2], F32, kind="ExternalOutput").ap()
    osg_d = nc.dram_tensor("o_sg", [128, F * 4], F32, kind="ExternalOutput").ap()

    with tile.TileContext(nc) as tc, ExitStack() as ctx:
        wp = ctx.enter_context(tc.tile_pool(name="w", bufs=1))
        iop = ctx.enter_context(tc.tile_pool(name="io", bufs=1))
        obp = ctx.enter_context(tc.tile_pool(name="ob", bufs=3))
        wk = ctx.enter_context(tc.tile_pool(name="wk", bufs=2))
        ek = ctx.enter_context(tc.tile_pool(name="ek", bufs=2))
        pp = ctx.enter_context(tc.tile_pool(name="pp", bufs=8, space="PSUM"))

        def wload(dram, shape, cvt=True):
            t = wp.tile(shape, F32, tag="wf_" + dram.tensor.name)
            nc.sync.dma_start(t[:], dram[:])
            if not cvt:
                return t
            r = wp.tile(shape, F32R, tag="wr_" + dram.tensor.name)
            nc.vector.tensor_copy(r[:], t[:])
            return r

        wd1 = wload(wd1_d, [4, 256], False)
        wd2 = wload(wd2_d, [128, 512], False)
        wd2r = wp.tile([128, 512], F32R, tag="wr_wd2")
        nc.vector.tensor_copy(wd2r[:], wd2[:])

        wd3 = wload(wd3_d, [128, 4]);   wm1 = wload(wm1_d, [66, 256])
        wm2 = wload(wm2_d, [128, 512]); wzr = wload(wzr_d, [128, 8])
        b1 = wload(b1_d, [128, 2], False);   b2 = wload(b2_d, [128, 2], False)
        bm1 = wload(bm1_d, [128, 2], False); bm2 = wload(bm2_d, [128, 2], False)
        bd3 = wload(bd3_d, [2, 1], False)
        bz2 = wload(bz2_d, [2, 1], False);   br2 = wload(br2_d, [2, 1], False)
        w1r = wload(w1r_d, [128, 4], False); qb = wload(qb_d, [128, 4], False)
        ident = wload(id_d, [128, 128], False)

        x1_sm = iop.tile([128, F * 4], F32)
        nc.sync.dma_start(x1_sm[:], x1_d[:])
        sg_sm = iop.tile([128, F * 4], F32)
        nc.sync.dma_start(sg_sm[:], sg_d[:])
        stag = iop.tile([128, F * Q10], F32)
        ost = iop.tile([128, F * 2], F32)
        osg = iop.tile([128, F * 4], F32)

        def chunk(m):
            obs_t = obp.tile([128, 256], F32, tag="obs")
            nc.sync.dma_start(obs_t[:], obs_d[:, m * 256:(m + 1) * 256])
            x1tp = pp.tile([4, 512], F32, tag="ps")
            obtp = pp.tile([64, 512], F32, tag="ps")
            for k in range(4):
                c = 4 * m + k
                nc.tensor.transpose(x1tp[:, k * 128:(k + 1) * 128],
                                    x1_sm[:, c * 4:(c + 1) * 4], ident[:])
                nc.tensor.transpose(obtp[:, k * 128:(k + 1) * 128],
                                    obs_t[:, k * 64:(k + 1) * 64], ident[:])
            x1t = wk.tile([4, 512], F32, tag="x1t")
            nc.vector.tensor_copy(x1t[:], x1tp[:])
            mrhs = wk.tile([66, 512], F32R, tag="mrhs")
            nc.scalar.copy(mrhs[0:64, :], obtp[:])

            h1, mk1 = [], []
            for t in range(2):
                p = pp.tile([128, 512], F32, tag="ps")
                nc.tensor.matmul(p[:], wd1[:, t * 128:(t + 1) * 128], x1t[:],
                                 start=True, stop=True)
                h = wk.tile([128, 512], F32, tag=f"h1_{t}")
                nc.scalar.activation(h[:], p[:], AF.Relu, bias=b1[:, t:t + 1])
                mk = wk.tile([128, 512], F32, tag=f"mk1_{t}")
                nc.vector.tensor_scalar(mk[:], h[:], 0.0, None, AL.is_gt)
                h1.append(h); mk1.append(mk)

            h2, mk2 = [], []
            for u in range(2):
                p = pp.tile([128, 512], F32, tag="ps")
                for kt in range(2):
                    nc.tensor.matmul(p[:], wd2[:, kt * 256 + u * 128: kt * 256 + (u + 1) * 128],
                                     h1[kt][:], start=(kt == 0), stop=(kt == 1))
                h = wk.tile([128, 512], F32R, tag=f"h2_{u}")
                nc.scalar.activation(h[:], p[:], AF.Relu, bias=b2[:, u:u + 1])
                mk = wk.tile([128, 512], F32, tag=f"mk2_{u}")
                nc.vector.tensor_scalar(mk[:], h[:].bitcast(F32), 0.0, None, AL.is_gt)
                h2.append(h); mk2.append(mk)

            d3p = pp.tile([2, 512], F32, tag="ps")
            for kt in range(2):
                nc.tensor.matmul(d3p[:], wd3[:, kt * 2:(kt + 1) * 2], h2[kt][:],
                                 start=(kt == 0), stop=(kt == 1))
            bnd = wk.tile([Q10, 512], F32, tag="bnd")
            nc.vector.scalar_tensor_tensor(bnd[0:2, :], d3p[:], bd3[:], x1t[0:2, :],
                                           AL.add, AL.add)
            nc.scalar.copy(mrhs[64:66, :], bnd[0:2, :])

            for i in range(2):
                t1 = []
                for t in range(2):
                    tt = wk.tile([128, 512], F32R, tag=f"t1_{i}_{t}")
                    nc.vector.tensor_scalar(tt[:], mk1[t][:], w1r[:, 2 * i + t:2 * i + t + 1],
                                            None, AL.mult)
                    t1.append(tt)
                t2 = []
                for u in range(2):
                    p = pp.tile([128, 512], F32, tag="ps")
                    for kt in range(2):
                        nc.tensor.matmul(p[:], wd2r[:, kt * 256 + u * 128: kt * 256 + (u + 1) * 128],
                                         t1[kt][:], start=(kt == 0), stop=(kt == 1))
                    tt2 = wk.tile([128, 512], F32R, tag=f"t2_{i}_{u}")
                    nc.vector.tensor_tensor(tt2[:], p[:], mk2[u][:], AL.mult)
                    t2.append(tt2)
                jp = pp.tile([2, 512], F32, tag="ps")
                for kt in range(2):
                    nc.tensor.matmul(jp[:], wd3[:, kt * 2:(kt + 1) * 2], t2[kt][:],
                                     start=(kt == 0), stop=(kt == 1))
                jq = wk.tile([2, 512], F32, tag=f"jq_{i}", name=f"jq_{i}")
                nc.vector.tensor_copy(jq[:], jp[:])
                nc.sync.dma_start(bnd[2 + 2 * i:4 + 2 * i, :], jq[:])

            g1 = []
            for t in range(2):
                p = pp.tile([128, 512], F32, tag="ps")
                nc.tensor.matmul(p[:], wm1[:, t * 128:(t + 1) * 128], mrhs[:],
                                 start=True, stop=True)
                g = wk.tile([128, 512], F32R, tag=f"g1_{t}")
                nc.scalar.activation(g[:], p[:], AF.Relu, bias=bm1[:, t:t + 1])
                g1.append(g)
            g2 = []
            for u in range(2):
                p = pp.tile([128, 512], F32, tag="ps")
                for kt in range(2):
                    nc.tensor.matmul(p[:], wm2[:, kt * 256 + u * 128: kt * 256 + (u + 1) * 128],
                                     g1[kt][:], start=(kt == 0), stop=(kt == 1))
                g = wk.tile([128, 512], F32R, tag=f"g2_{u}")
                nc.scalar.activation(g[:], p[:], AF.Relu, bias=bm2[:, u:u + 1])
                g2.append(g)
            zp = pp.tile([2, 512], F32, tag="ps")
            rp = pp.tile([2, 512], F32, tag="ps")
            for kt in range(2):
                nc.tensor.matmul(zp[:], wzr[:, kt * 4:kt * 4 + 2], g2[kt][:],
                                 start=(kt == 0), stop=(kt == 1))
            for kt in range(2):
                nc.tensor.matmul(rp[:], wzr[:, kt * 4 + 2:(kt + 1) * 4], g2[kt][:],
                                 start=(kt == 0), stop=(kt == 1))
            zq = wk.tile([2, 512], F32, tag="zq")
            nc.vector.tensor_scalar(zq[:], zp[:], bz2[:], None, AL.add)
            nc.sync.dma_start(bnd[6:8, :], zq[:])
            # r = softplus(pre_r)+1e-3, pre_r = rp+br; softplus = max(x,0)+ln(1+exp(-|x|))
            axt = wk.tile([2, 512], F32, tag="axt")
            nc.scalar.activation(axt[:], rp[:], AF.Abs, bias=br2[:])
            ext = wk.tile([2, 512], F32, tag="ext")
            nc.scalar.activation(ext[:], axt[:], AF.Exp, scale=-1.0)
            lnt = wk.tile([2, 512], F32, tag="lnt")
            nc.scalar.activation(lnt[:], ext[:], AF.Ln, bias=1.0)
            mxt = wk.tile([2, 512], F32, tag="mxt")
            nc.vector.tensor_scalar(mxt[:], rp[:], br2[:], 0.0, AL.add, AL.max)
            rq = wk.tile([2, 512], F32, tag="rq")
            nc.vector.scalar_tensor_tensor(rq[:], lnt[:], 1e-3, mxt[:], AL.add, AL.add)
            nc.sync.dma_start(bnd[8:10, :], rq[:])

            bndt = pp.tile([128, 4 * Q10], F32, tag="ps")
            for k in range(4):
                nc.tensor.transpose(bndt[:, k * Q10:(k + 1) * Q10],
                                    bnd[:, k * 128:(k + 1) * 128], ident[0:Q10, 0:Q10])
            nc.vector.tensor_copy(stag[:, m * 4 * Q10:(m + 1) * 4 * Q10], bndt[:])

        def ekf():
            stv = stag[:].rearrange("p (c q) -> p c q", q=Q10)
            sgv = sg_sm[:].rearrange("p (f e) -> p f e", e=4)
            ostv = ost[:].rearrange("p (c l) -> p c l", l=2)
            osgv = osg[:].rearrange("p (c l) -> p c l", l=4)
            S = lambda q: stv[:, :, q:q + 1]
            P = lambda e: sgv[:, :, e:e + 1]
            sc = {}

            def T(name):
                if name not in sc:
                    sc[name] = ek.tile([128, F, 1], F32, tag="e_" + name, name="e_" + name)
                return sc[name][:]

            V = nc.vector
            sp0, sp1 = S(0), S(1)
            j00, j10, j01, j11 = S(2), S(3), S(4), S(5)
            z0, z1, r0, r1 = S(6), S(7), S(8), S(9)
            p00, p01, p10, p11 = P(0), P(1), P(2), P(3)
            a00, a11 = T("a00"), T("a11")
            V.tensor_scalar(a00, j00, 1.0, None, AL.add)
            V.tensor_scalar(a11, j11, 1.0, None, AL.add)
            t, u = T("t"), T("u")
            # AP = A @ P
            ap00, ap01, ap10, ap11 = T("ap00"), T("ap01"), T("ap10"), T("ap11")
            for (o, aL, pL, aR, pR) in ((ap00, a00, p00, j01, p10), (ap01, a00, p01, j01, p11),
                                        (ap10, j10, p00, a11, p10), (ap11, j10, p01, a11, p11)):
                V.tensor_tensor(t, aL, pL, AL.mult)
                V.tensor_tensor(u, aR, pR, AL.mult)
                V.tensor_tensor(o, t, u, AL.add)
            # Sigma_pred = AP @ A^T + Q
            s00, s01, s10, s11 = T("s00"), T("s01"), T("s10"), T("s11")
            for (o, x1_, aL, x2_, aR, qi) in (
                    (s00, ap00, a00, ap01, j01, 0), (s01, ap00, j10, ap01, a11, 1),
                    (s10, ap10, a00, ap11, j01, 2), (s11, ap10, j10, ap11, a11, 3)):
                V.tensor_tensor(t, x1_, aL, AL.mult)
                V.tensor_tensor(u, x2_, aR, AL.mult)
                V.scalar_tensor_tensor(o, t, qb[:, qi:qi + 1], u, AL.add, AL.add)
            # S = Sigma + R (diag), inverse via adjugate
            e00, e11 = T("e00"), T("e11")
            V.tensor_tensor(e00, s00, r0, AL.add)
            V.tensor_tensor(e11, s11, r1, AL.add)
            det, rdet = T("det"), T("rdet")
            V.tensor_tensor(t, e00, e11, AL.mult)
            V.tensor_tensor(u, s01, s10, AL.mult)
            V.tensor_tensor(det, t, u, AL.subtract)
            V.reciprocal(rdet, det)
            # K = Sigma @ inv(S)
            k00, k01, k10, k11 = T("k00"), T("k01"), T("k10"), T("k11")
            for (o, x1_, aL, x2_, aR) in (
                    (k00, s00, e11, s01, s10), (k01, s01, e00, s00, s01),
                    (k10, s10, e11, s11, s10), (k11, s11, e00, s10, s01)):
                V.tensor_tensor(t, x1_, aL, AL.mult)
                V.tensor_tensor(u, x2_, aR, AL.mult)
                V.tensor_tensor(o, t, u, AL.subtract)
                V.tensor_tensor(o, o, rdet, AL.mult)
            y0, y1 = T("y0"), T("y1")
            V.tensor_tensor(y0, z0, sp0, AL.subtract)
            V.tensor_tensor(y1, z1, sp1, AL.subtract)
            for (oc, ka, kb, spx) in ((0, k00, k01, sp0), (1, k10, k11, sp1)):
                V.tensor_tensor(t, ka, y0, AL.mult)
                V.tensor_tensor(u, kb, y1, AL.mult)
                V.tensor_tensor(t, t, u, AL.add)
                V.tensor_tensor(ostv[:, :, oc:oc + 1], spx, t, AL.add)
            ik00, ik01, ik10, ik11 = T("ik00"), T("ik01"), T("ik10"), T("ik11")
            V.tensor_scalar(ik00, k00, -1.0, 1.0, AL.mult, AL.add)
            V.tensor_scalar(ik11, k11, -1.0, 1.0, AL.mult, AL.add)
            V.tensor_scalar(ik01, k01, -1.0, None, AL.mult)
            V.tensor_scalar(ik10, k10, -1.0, None, AL.mult)
            # Sigma_update = (I-K) @ Sigma ; out[0,1] forced = out[1,0]
            su10 = T("su10")
            for (o, ka, xa, kb, xb) in (
                    (osgv[:, :, 0:1], ik00, s00, ik01, s10),
                    (su10, ik10, s00, ik11, s10),
                    (osgv[:, :, 3:4], ik10, s01, ik11, s11)):
                V.tensor_tensor(t, ka, xa, AL.mult)
                V.tensor_tensor(u, kb, xb, AL.mult)
                V.tensor_tensor(o, t, u, AL.add)
            V.tensor_copy(osgv[:, :, 1:2], su10)
            V.tensor_copy(osgv[:, :, 2:3], su10)

        def body():
            for m in range(NCHUNK):
                chunk(m)
            ekf()
            nc.sync.dma_start(ost_d[:], ost[:])
            nc.sync.dma_start(osg_d[:], osg[:])

        if reps == 1:
            body()
        else:
            with tc.For_i(0, reps, 1):
                body()

    nc.compile()
    _built[reps] = nc
    return nc


def _prep_shared(Wd1, bd1, Wd2, bd2, Wd3, bd3, Wm1, bm1, Wm2, bm2, Wz, bz, Wr, br, Q):
    f = np.float32
    out = {
        "wd1": np.ascontiguousarray(Wd1, f),
        "wd2": np.ascontiguousarray(Wd2.reshape(2, 128, 256).transpose(1, 0, 2).reshape(128, 512), f),
        "wd3": np.ascontiguousarray(Wd3.reshape(2, 128, 2).transpose(1, 0, 2).reshape(128, 4), f),
        "wm1": np.ascontiguousarray(Wm1, f),
        "wm2": np.ascontiguousarray(Wm2.reshape(2, 128, 256).transpose(1, 0, 2).reshape(128, 512), f),
        "wzr": np.ascontiguousarray(np.concatenate([Wz, Wr], 1).reshape(2, 128, 4).transpose(1, 0, 2).reshape(128, 8), f),
        "b1": np.ascontiguousarray(bd1.reshape(2, 128).T, f),
        "b2": np.ascontiguousarray(bd2.reshape(2, 128).T, f),
        "bm1": np.ascontiguousarray(bm1.reshape(2, 128).T, f),
        "bm2": np.ascontiguousarray(bm2.reshape(2, 128).T, f),
        "bd3": np.ascontiguousarray(bd3.reshape(2, 1), f),
        "bz2": np.ascontiguousarray(bz.reshape(2, 1), f),
        "br2": np.ascontiguousarray(br.reshape(2, 1), f),
        "w1r": np.ascontiguousarray(
            np.stack([Wd1[0, :128], Wd1[0, 128:], Wd1[1, :128], Wd1[1, 128:]], 1), f),
        "qb": np.ascontiguousarray(np.tile(np.asarray(Q, f).reshape(1, 4), (128, 1))),
        "ident": np.eye(128, dtype=f),
    }
    return out


def run(reps, states_prev, states_sigma_prev, observations, controls,
        Wd1, bd1, Wd2, bd2, Wd3, bd3, Wm1, bm1, Wm2, bm2, Wz, bz, Wr, br, Q):
    from concourse import bass_utils
    nc = _build(reps)
    shared = _prep_shared(Wd1, bd1, Wd2, bd2, Wd3, bd3, Wm1, bm1, Wm2, bm2, Wz, bz, Wr, br, Q)
    f = np.float32
    sp = np.asarray(states_prev, f)
    sg = np.asarray(states_sigma_prev, f).reshape(N, 4)
    ob = np.asarray(observations, f)
    ct = np.asarray(controls, f)
    in_maps = []
    for c in range(NCORES):
        lo, hi = c * NC_SAMP, (c + 1) * NC_SAMP
        m = dict(shared)
        m["x1"] = np.ascontiguousarray(
            np.concatenate([sp[lo:hi], ct[lo:hi]], 1).reshape(128, F * 4))
        m["obs"] = np.ascontiguousarray(ob[lo:hi].reshape(128, F * 64))
        m["sg"] = np.ascontiguousarray(sg[lo:hi].reshape(128, F * 4))
        in_maps.append(m)
    import time
    t0 = time.time()
    res = bass_utils.run_bass_kernel_spmd(nc, in_maps, core_ids=list(range(NCORES)), trace=False)
    wall = time.time() - t0
    st = np.concatenate([res.results[c]["o_st"].reshape(NC_SAMP, 2) for c in range(NCORES)], 0)
    sgo = np.concatenate([res.results[c]["o_sg"].reshape(NC_SAMP, 2, 2) for c in range(NCORES)], 0)
    return (st, sgo), wall


def kernel(**inputs):
    (st, sgo), _ = run(1, **inputs)
    return st, sgo
